# revision 2
# baseline (speedup 1.0000x reference)
"""Multi-head attention (b=2, n=2048, d=1024, H=16 heads) on 8 TRN2 NeuronCores.

Sharding: core c = (b, g) with b = c // 4 (data parallel over batch) and
g = c % 4 (tensor parallel over head groups of 4 heads).  Each core computes
qkv projections for its 4 heads, full softmax attention for those heads, and
a partial output projection y_partial = A_heads @ w_out[g*256:(g+1)*256].
The host sums the 4 partials per batch (bf16 partials, f32 accumulate) and
adds b_out.

v2 redesign: the kernel is one flat software-pipelined stream of 128 steps
(8 blocks of (q-chunk, head-pair) x 16 nk-tiles).  Per step s the PE emission
order is [scores(s+1) pair, PV0(s), PV1(s), filler pieces]; the exp for step
s runs on ScalarE (the only engine with Exp; ~1.0us per [128,1024] tile).
Scores(s+1) depends only on exp(s-1) (PSUM WAR through the 2-deep st
rotation), so ScalarE runs back-to-back while the PE fills its exp-wait with
the score pair for the next step plus one piece of projection work
(qkv / out-proj), keeping both engines busy concurrently.

PSUM budget (8 banks): st double-buffer 2x2 + o accumulators 2 + filler 2.
Fillers (kq units 8mm/4 pieces, v units 8mm/2 pieces, proj units 2mm) are
emitted deadline-first then paced uniformly; the greedy tile scheduler pops
them whenever the PE would otherwise idle.

Softmax denominators: DVE reciprocal is an iterative ~8-cycle/element op
whose cost scales with per-lane (free-dim) element count, so reciprocal of a
[1,512] denominator row costs ~2us and head-of-line-blocks the DVE queue
(which stalls the kq-unit copies the next block's scores wait on).  Instead,
each chunk's 4 denominator rows are DMA-reshaped into a [32,4,16] tile
(elements spread across partitions), reciprocal'd in ONE ~0.7us op, DMA-
reshaped back to [1,512] rows, partition-broadcast on the (otherwise idle)
GpSimd engine, and multiplied into A^T bf16 on DVE.  half1 of each pair is
shifted to partitions 64-127 via SBUF->SBUF DMA.

y is stored bf16 (halves both the PSUM->SBUF copy cost and the HBM write);
the host accumulates partials in f32.  Measured end-to-end relative error
~5e-3 vs the fp32 reference.
"""

import os
import sys

for _p in ("/opt/trn_rl_repo",):
    if _p not in sys.path and os.path.isdir(_p):
        sys.path.insert(0, _p)

import ml_dtypes
import numpy as np

import concourse.bass as bass
import concourse.mybir as mybir
import concourse.tile as tile
from concourse import bacc

P = 128
D = 1024          # model dim
N = 2048          # sequence length
HD = 64           # head dim
GH = 4            # heads per core
DG = GH * HD      # 256 projected cols per core
KD = D // P       # 8 k-tiles over model dim
NT = N // P       # 16 tiles over sequence
QC = 512          # n_q chunk size
NQC = N // QC     # 4 chunks
NSTEP = 8 * NT    # 128 pipeline steps
SCALE = HD ** -0.5

F32 = mybir.dt.float32
BF16 = mybir.dt.bfloat16

Exp = mybir.ActivationFunctionType.Exp

# step s -> (block, t); blocks in (chunk, pair) order so out-proj for chunk c
# unlocks as early as possible (after block (c, 1))
BLOCKS = [(c, pr) for c in range(NQC) for pr in range(2)]


def build_nc():
    nc = bacc.Bacc("TRN2")

    xt = nc.declare_dram_parameter("xt", [D, N], BF16, isOutput=False)
    wq = nc.declare_dram_parameter("wq", [D, DG], BF16, isOutput=False)
    wk = nc.declare_dram_parameter("wk", [D, DG], BF16, isOutput=False)
    wv = nc.declare_dram_parameter("wv", [D, DG], BF16, isOutput=False)
    wo = nc.declare_dram_parameter("wo", [DG, D], BF16, isOutput=False)
    y = nc.declare_dram_parameter("y", [N, D], BF16, isOutput=True)

    xt_r = xt[:, :].rearrange("(o p) n -> p o n", p=P)    # [128, 8, 2048]
    wq_r = wq[:, :].rearrange("(o p) n -> p o n", p=P)    # [128, 8, 256]
    wk_r = wk[:, :].rearrange("(o p) n -> p o n", p=P)
    wv_r = wv[:, :].rearrange("(o p) n -> p o n", p=P)
    wo_r = wo[:, :].rearrange("(o p) n -> p o n", p=P)    # [128, 2, 1024]
    y_r = y[:, :].rearrange("(o p) n -> p o n", p=P)      # [128, 16, 1024]

    with tile.TileContext(nc) as tc, nc.allow_low_precision("bf16 attention"):
        with (
            tc.tile_pool(name="wpool", bufs=1) as wpool,
            tc.tile_pool(name="qkvpool", bufs=1) as qkvpool,
            tc.tile_pool(name="xpool", bufs=1) as xpool,
            tc.tile_pool(name="epool", bufs=6) as epool,
            tc.tile_pool(name="work", bufs=2) as work,
            tc.tile_pool(name="opool", bufs=4) as opool,
            tc.tile_pool(name="rcpool", bufs=4) as rcpool,
            tc.tile_pool(name="outp", bufs=2) as outp,
            tc.tile_pool(name="ps_st", bufs=2, space="PSUM") as ps_st,
            tc.tile_pool(name="ps_o", bufs=2, space="PSUM") as ps_o,
            tc.tile_pool(name="ps_f", bufs=2, space="PSUM") as ps_f,
        ):
            # ---------------- input DMAs, in consumption order ----------------
            wk_sb = wpool.tile([P, KD, DG], BF16, tag="wk")
            wq_sb = wpool.tile([P, KD, DG], BF16, tag="wq")
            wv_sb = wpool.tile([P, KD, DG], BF16, tag="wv")
            wo_sb = wpool.tile([P, 2, D], BF16, tag="wo")
            xt_sb = xpool.tile([P, KD, N], BF16, tag="xt")

            # few BIG 3D DMAs (each DMA_DIRECT2D occupies its issuing
            # queue ~0.65us, so issue count matters).  The bulk of x is
            # pinned BEHIND the critical first 2.5MB (wk+xc0+wq+wv) with an
            # explicit dependency so it doesn't steal HBM bandwidth from the
            # transfers that gate the first exp.
            # staged, ring-parallel input DMAs.  One DMA ring moves only
            # ~100-200GB/s, so the first-exp-critical 2MB (wk+xc0+wq) is
            # split across ~6 rings; later stages are gated behind earlier
            # ones with explicit deps so they never steal HBM bandwidth
            # from the transfer that gates the pipeline next.
            s1 = [
                nc.sync.dma_start(wk_sb[:, 0:4, :], wk_r[:, 0:4, :]),
                nc.sync.dma_start(xt_sb[:, 0:2, 0:QC], xt_r[:, 0:2, 0:QC]),
                nc.sync.dma_start(xt_sb[:, 2:4, 0:QC], xt_r[:, 2:4, 0:QC]),
                nc.sync.dma_start(wk_sb[:, 4:KD, :], wk_r[:, 4:KD, :]),
                nc.sync.dma_start(xt_sb[:, 4:6, 0:QC], xt_r[:, 4:6, 0:QC]),
                nc.sync.dma_start(xt_sb[:, 6:KD, 0:QC], xt_r[:, 6:KD, 0:QC]),
            ]
            s2 = [
                nc.sync.dma_start(wq_sb[:, 0:4, :], wq_r[:, 0:4, :]),
                nc.sync.dma_start(wq_sb[:, 4:KD, :], wq_r[:, 4:KD, :]),
            ]
            s3 = [
                nc.gpsimd.dma_start(wv_sb[:], wv_r),
                nc.gpsimd.dma_start(xt_sb[:, :, QC:10 * P], xt_r[:, :, QC:10 * P]),
            ]
            s4 = [
                nc.gpsimd.dma_start(xt_sb[:, :, 10 * P:N], xt_r[:, :, 10 * P:N]),
                nc.gpsimd.dma_start(wo_sb[:], wo_r),
            ]
            for stage, prev in ((s2, s1[-1]), (s3, s2[-1]), (s4, s3[-1])):
                for d in stage:
                    bass._add_dep_helper(
                        d.ins, prev.ins, sync=True, reason="staged input dma"
                    )

            # ---------------- persistent tensors ----------------
            qt_sb = qkvpool.tile([P, 2, N], BF16, tag="qt")   # [256, 2048] qT
            kt_sb = qkvpool.tile([P, 2, N], BF16, tag="kt")   # [256, 2048] kT
            vg_sb = qkvpool.tile([P, NT, GH, 66], BF16, tag="vg")  # v + ones
            nc.scalar.copy(
                vg_sb[:, :, :, HD:], nc.const_aps.tensor(1.0, (P, NT, GH, 2), F32)
            )
            at_sb = qkvpool.tile([P, 2, N], BF16, tag="at")   # attn_outT

            # ---------------- filler units ----------------
            # Each filler unit is a list of pieces; a piece is a closure that
            # emits ~2 matmuls (and, on a unit's last piece, the PSUM->SBUF
            # copy / DMA).  Units alternate between the two ps_f banks.

            def kq_unit(which, w_sb, dst, m, cchunk):
                state = {}
                cs = slice(cchunk * QC, (cchunk + 1) * QC)

                def piece(i0):
                    def go():
                        if i0 == 0:
                            state["ps"] = ps_f.tile(
                                [P, QC], F32, tag="f",
                                name=f"{which}ps_{m}_{cchunk}",
                            )
                        ps = state["ps"]
                        for k in range(i0, i0 + 2):
                            nc.tensor.matmul(
                                ps[:],
                                w_sb[:, k, m * P:(m + 1) * P],
                                xt_sb[:, k, cs],
                                start=(k == 0),
                                stop=(k == KD - 1),
                            )
                        if i0 + 2 == KD:
                            nc.vector.tensor_copy(dst[:, m, cs], ps[:])
                    return go

                return [piece(i) for i in range(0, KD, 2)]

            def v_unit(t):
                state = {}

                def piece(i0):
                    def go():
                        if i0 == 0:
                            state["ps"] = ps_f.tile(
                                [P, QC], F32, tag="f", name=f"vps_{t}"
                            )
                        ps = state["ps"]
                        for k in range(i0, i0 + 4):
                            nc.tensor.matmul(
                                ps[:, :DG],
                                xt_sb[:, k, t * P:(t + 1) * P],
                                wv_sb[:, k, :],
                                start=(k == 0),
                                stop=(k == KD - 1),
                            )
                        if i0 + 4 == KD:
                            nc.vector.tensor_copy(
                                vg_sb[:, t, :, 0:HD],
                                ps[:, :DG].rearrange("p (h e) -> p h e", h=GH),
                            )
                    return go

                return [piece(i) for i in range(0, KD, 4)]

            tail_n = [0]

            def proj_unit(m, nn, tail=False):
                def go():
                    if tail:
                        # ps_o's banks are free once the last block is
                        # staged; alternating pools gives the tail proj a
                        # 4-deep PSUM rotation
                        pool = ps_o if tail_n[0] % 2 else ps_f
                        tail_n[0] += 1
                        ps = pool.tile(
                            [P, QC], F32, tag="o" if pool is ps_o else "f",
                            name=f"yps_{m}_{nn}",
                        )
                    else:
                        ps = ps_f.tile([P, QC], F32, tag="f", name=f"yps_{m}_{nn}")
                    for ks in range(2):
                        nc.tensor.matmul(
                            ps[:],
                            at_sb[:, ks, m * P:(m + 1) * P],
                            wo_sb[:, ks, nn * QC:(nn + 1) * QC],
                            start=(ks == 0),
                            stop=(ks == 1),
                        )
                    ysb = outp.tile([P, QC], BF16, tag="y", name=f"y_{m}_{nn}")
                    if tail:
                        # ScalarE is idle once the last exp is done, while
                        # DVE is busy with the tail reciprocals
                        nc.scalar.copy(ysb[:], ps[:])
                    else:
                        nc.vector.tensor_copy(ysb[:], ps[:])
                    if tail:
                        nc.gpsimd.dma_start(y_r[:, m, nn * QC:(nn + 1) * QC], ysb[:])
                    else:
                        nc.sync.dma_start(y_r[:, m, nn * QC:(nn + 1) * QC], ysb[:])
                return [go]

            # (deadline, pieces) — deadline = step by which the unit should be
            # emitted (need-step minus ~2-3 of execution-lag lead time); proj
            # units are appended with deadline None (pace-driven)
            fillers = []
            for t in range(1, NT):
                fillers.append((t - 2, v_unit(t)))
            for cc in range(1, NQC):
                fillers.append((4 * cc - 4, kq_unit("k", wk_sb, kt_sb, 0, cc)))
            for cc in range(NQC):
                fillers.append((12 + 4 * cc, kq_unit("k", wk_sb, kt_sb, 1, cc)))
            fillers.append((12, kq_unit("q", wq_sb, qt_sb, 1, 0)))
            for cc in range(1, NQC):
                fillers.append((32 * cc - 4, kq_unit("q", wq_sb, qt_sb, 0, cc)))
                fillers.append((32 * cc + 12, kq_unit("q", wq_sb, qt_sb, 1, cc)))
            fillers.sort(key=lambda f: f[0])
            fill_q = [(d, p) for d, pieces in fillers for p in pieces]
            total_pieces = len(fill_q) + 32  # + proj pieces appended later
            emitted = [0]

            def emit_fillers(s):
                # everything whose deadline has arrived, then pace the rest
                while fill_q and fill_q[0][0] is not None and fill_q[0][0] <= s:
                    fill_q.pop(0)[1]()
                    emitted[0] += 1
                pace = (s + 1) * total_pieces // NSTEP
                while fill_q and emitted[0] < pace:
                    fill_q.pop(0)[1]()
                    emitted[0] += 1

            # ---------------- attention stream helpers ----------------
            st_tiles = {}
            e_tiles = {}

            def emit_sc(s):
                b, t = s // NT, s % NT
                c, pr = BLOCKS[b]
                cs = slice(c * QC, (c + 1) * QC)
                ts_ = slice(t * P, (t + 1) * P)
                st = ps_st.tile([P, 2, QC], F32, tag="st", name=f"st_{s}")
                for half in range(2):
                    hs = slice(half * HD, (half + 1) * HD)
                    nc.tensor.matmul(
                        st[:, half, :],
                        kt_sb[hs, pr, ts_],
                        qt_sb[hs, pr, cs],
                        start=True,
                        stop=True,
                    )
                st_tiles[s] = st

            def emit_exp(s):
                e = epool.tile([P, 2, QC], BF16, tag="e", name=f"e_{s}")
                nc.scalar.activation(e[:], st_tiles.pop(s)[:], Exp, scale=SCALE)
                e_tiles[s] = e

            def emit_pv(s, o_ps):
                b, t = s // NT, s % NT
                _, pr = BLOCKS[b]
                e = e_tiles.pop(s)
                for half in range(2):
                    h = 2 * pr + half
                    nc.tensor.matmul(
                        o_ps[half][:],
                        vg_sb[:, t, h, 0:HD + 1],
                        e[:, half, :],
                        start=(t == 0),
                        stop=(t == NT - 1),
                    )

            # ---------------- softmax normalization ----------------
            # Per-block: stage the two [65,512] accumulators to SBUF (frees
            # the PSUM banks) and DMA-reshape the denominator rows [1,512]
            # into partition-spread [32,16] slots of the chunk's den tile.
            # Per-chunk: ONE reciprocal over [32,4,16] (iterative-divide cost
            # scales with per-lane elements: 64 here vs 512 for a row),
            # DMA-reshape each slot back to a [1,512] row, partition-
            # broadcast on GpSimd, multiply into A^T on DVE.
            chunk_state = {}

            def emit_stage_and_norm(b):
                # stage this block's accumulators, spread its two denominator
                # rows across partitions, reciprocal, and normalize this
                # pair's A^T rows right away (per-pair, so only half the
                # chain lands after the final block)
                c, pr = BLOCKS[b]
                if pr == 0:
                    den = work.tile([32, 4, 16], F32, tag="den", name=f"den_{c}")
                    rcT = work.tile([32, 4, 16], F32, tag="rcT", name=f"rcT_{c}")
                    chunk_state[c] = {"den": den, "rcT": rcT}
                st_ = chunk_state[c]
                den, rcT = st_["den"], st_["rcT"]
                o_sbs = []
                for half in range(2):
                    o_sb = opool.tile(
                        [HD + 1, QC], F32, tag="osb", name=f"osb_{b}_{half}"
                    )
                    nc.vector.tensor_copy(o_sb[:], o_tiles[b][half][:])
                    o_sbs.append(o_sb)
                    slot = 2 * pr + half
                    nc.sync.dma_start(den[:, slot, :], o_sb[HD:HD + 1, :])
                nc.vector.reciprocal(
                    rcT[:, 2 * pr:2 * pr + 2, :], den[:, 2 * pr:2 * pr + 2, :]
                )
                for half in range(2):
                    o_sb = o_sbs[half]
                    slot = 2 * pr + half
                    rcr = rcpool.tile(
                        [1, QC], F32, tag="rcr", name=f"rcr_{c}_{slot}"
                    )
                    nc.sync.dma_start(rcr[:], rcT[:, slot, :])
                    rbs = rcpool.tile(
                        [HD, QC], F32, tag="rbs", name=f"rbs_{c}_{slot}"
                    )
                    nc.gpsimd.partition_broadcast(rbs[:], rcr[:])
                    dst = slice(c * QC, (c + 1) * QC)
                    if half == 0:
                        nc.vector.tensor_mul(
                            at_sb[0:HD, pr, dst], o_sb[0:HD, :], rbs[:]
                        )
                    else:
                        stg = work.tile(
                            [HD, QC], BF16, tag="stg", name=f"stg_{c}_{pr}"
                        )
                        nc.vector.tensor_mul(stg[:], o_sb[0:HD, :], rbs[:])
                        nc.sync.dma_start(at_sb[HD:P, pr, dst], stg[:])
                if pr == 1:
                    chunk_state.pop(c)

            # ---------------- prologue ----------------
            # ~3.5us of dummy matmuls on constants while the first DMAs are
            # in flight: the PE's HAM clock-gate needs ~3.4us of sustained
            # activity to unthrottle 1.2->2.4GHz, so the first REAL matmuls
            # run at full speed instead of paying the cold tax
            warm_c = nc.const_aps.tensor(1.0, (P, QC), BF16)
            for wi in range(8):
                wps = ps_f.tile([P, QC], F32, tag="f", name=f"warm_{wi}")
                nc.tensor.matmul(
                    wps[:], warm_c[:, 0:P], warm_c[:], start=True, stop=True
                )
            for p in kq_unit("k", wk_sb, kt_sb, 0, 0):
                p()
            for p in kq_unit("q", wq_sb, qt_sb, 0, 0):
                p()
            for p in v_unit(0):
                p()

            # ---------------- the 128-step stream ----------------
            o_tiles = {}

            def alloc_o(b):
                o_ps = []
                for half in range(2):
                    o_full = ps_o.tile([P, QC], F32, tag="o", name=f"o_{b}_{half}")
                    o_ps.append(o_full[: HD + 1])
                o_tiles[b] = o_ps

            emit_sc(0)
            for s in range(NSTEP):
                b, t = s // NT, s % NT
                if t == 0:
                    alloc_o(b)
                if s + 1 < NSTEP:
                    emit_sc(s + 1)
                emit_exp(s)
                emit_pv(s, o_tiles[b])
                if t == NT - 1:
                    c, pr = BLOCKS[b]
                    if b < len(BLOCKS) - 1:
                        emit_stage_and_norm(b)
                        if pr == 1:
                            for mi in range(4):
                                for nn in range(2):
                                    fill_q.append(
                                        (None, proj_unit(4 * c + mi, nn)[0])
                                    )
                emit_fillers(s)
            # tail: last block's normalization via the direct (low-latency)
            # reciprocal path, column-split and interleaved with the last
            # chunk's proj units so the PE restarts as early as possible
            b = len(BLOCKS) - 1
            c, pr = BLOCKS[b]
            o_sbs = []
            for half in range(2):
                o_sb = opool.tile(
                    [HD + 1, QC], F32, tag="osb", name=f"osb_{b}_{half}"
                )
                # ScalarE is idle after the last exp; keep DVE free for the
                # reciprocals
                nc.scalar.copy(o_sb[:], o_tiles[b][half][:])
                o_sbs.append(o_sb)
            for sp in range(2):
                w = QC // 2
                ss = slice(sp * w, (sp + 1) * w)
                for half in range(2):
                    o_sb = o_sbs[half]
                    rc = rcpool.tile([1, w], F32, tag="rcd", name=f"rcd_{half}_{sp}")
                    nc.vector.reciprocal(rc[:], o_sb[HD:HD + 1, ss])
                    rbs = rcpool.tile([HD, w], F32, tag="rbsd", name=f"rbsd_{half}_{sp}")
                    nc.gpsimd.partition_broadcast(rbs[:], rc[:])
                    dst = slice(c * QC + sp * w, c * QC + (sp + 1) * w)
                    if half == 0:
                        nc.vector.tensor_mul(
                            at_sb[0:HD, pr, dst], o_sb[0:HD, ss], rbs[:]
                        )
                    else:
                        stg = work.tile(
                            [HD, w], BF16, tag="stgd", name=f"stgd_{sp}"
                        )
                        nc.vector.tensor_mul(stg[:], o_sb[0:HD, ss], rbs[:])
                        nc.sync.dma_start(at_sb[HD:P, pr, dst], stg[:])
                for mi in (2 * sp, 2 * sp + 1):
                    for nn in range(2):
                        fill_q.append(
                            (None, proj_unit(4 * c + mi, nn, tail=True)[0])
                        )
                while fill_q:
                    fill_q.pop(0)[1]()

    nc.finalize()
    return nc


_NC = None


def _get_nc():
    global _NC
    if _NC is None:
        _NC = build_nc()
    return _NC


def _in_maps(x, w_qkv, w_out):
    bf = ml_dtypes.bfloat16
    x = np.asarray(x, dtype=np.float32)
    w_qkv = np.asarray(w_qkv, dtype=np.float32)
    w_out = np.asarray(w_out, dtype=np.float32)
    xts = [np.ascontiguousarray(x[b].T).astype(bf) for b in range(2)]
    wq_g = [np.ascontiguousarray(w_qkv[:, 0 * D + g * DG:0 * D + (g + 1) * DG]).astype(bf) for g in range(4)]
    wk_g = [np.ascontiguousarray(w_qkv[:, 1 * D + g * DG:1 * D + (g + 1) * DG]).astype(bf) for g in range(4)]
    wv_g = [np.ascontiguousarray(w_qkv[:, 2 * D + g * DG:2 * D + (g + 1) * DG]).astype(bf) for g in range(4)]
    wo_g = [np.ascontiguousarray(w_out[g * DG:(g + 1) * DG, :]).astype(bf) for g in range(4)]
    maps = []
    for c in range(8):
        b, g = c // 4, c % 4
        maps.append({
            "xt": xts[b],
            "wq": wq_g[g],
            "wk": wk_g[g],
            "wv": wv_g[g],
            "wo": wo_g[g],
        })
    return maps


LAST_RESULT = None


def kernel(x, w_qkv, w_out, b_out):
    from concourse.bass_utils import run_bass_kernel_spmd

    nc = _get_nc()
    maps = _in_maps(x, w_qkv, w_out)
    res = run_bass_kernel_spmd(nc, maps, list(range(8)))
    global LAST_RESULT
    LAST_RESULT = res
    out = np.zeros((2, N, D), dtype=np.float32)
    for c in range(8):
        out[c // 4] += np.asarray(res.results[c]["y"], dtype=np.float32)
    out += np.asarray(b_out, dtype=np.float32)[None, None, :]
    return out


# revision 3
# speedup vs baseline: 1.0419x; 1.0419x over previous
"""Multi-head attention (b=2, n=2048, d=1024, H=16 heads) on 8 TRN2 NeuronCores.

Sharding: core c = (b, g) with b = c // 4 (data parallel over batch) and
g = c % 4 (tensor parallel over head groups of 4 heads).  Each core computes
qkv projections for its 4 heads, full softmax attention for those heads, and
a partial output projection y_partial = A_heads @ w_out[g*256:(g+1)*256].
The host sums the 4 partials per batch (bf16 partials, f32 accumulate) and
adds b_out.

v2 redesign: the kernel is one flat software-pipelined stream of 128 steps
(8 blocks of (q-chunk, head-pair) x 16 nk-tiles).  Per step s the PE emission
order is [scores(s+1) pair, PV0(s), PV1(s), filler pieces]; the exp for step
s runs on ScalarE (the only engine with Exp; ~1.0us per [128,1024] tile).
Scores(s+1) depends only on exp(s-1) (PSUM WAR through the 2-deep st
rotation), so ScalarE runs back-to-back while the PE fills its exp-wait with
the score pair for the next step plus one piece of projection work
(qkv / out-proj), keeping both engines busy concurrently.

PSUM budget (8 banks): st double-buffer 2x2 + o accumulators 2 + filler 2.
Fillers (kq units 8mm/4 pieces, v units 8mm/2 pieces, proj units 2mm) are
emitted deadline-first then paced uniformly; the greedy tile scheduler pops
them whenever the PE would otherwise idle.

Softmax denominators: DVE reciprocal is an iterative ~8-cycle/element op
whose cost scales with per-lane (free-dim) element count, so reciprocal of a
[1,512] denominator row costs ~2us and head-of-line-blocks the DVE queue
(which stalls the kq-unit copies the next block's scores wait on).  Instead,
each chunk's 4 denominator rows are DMA-reshaped into a [32,4,16] tile
(elements spread across partitions), reciprocal'd in ONE ~0.7us op, DMA-
reshaped back to [1,512] rows, partition-broadcast on the (otherwise idle)
GpSimd engine, and multiplied into A^T bf16 on DVE.  half1 of each pair is
shifted to partitions 64-127 via SBUF->SBUF DMA.

y is stored bf16 (halves both the PSUM->SBUF copy cost and the HBM write);
the host accumulates partials in f32.  Measured end-to-end relative error
~5e-3 vs the fp32 reference.
"""

import os
import sys

for _p in ("/opt/trn_rl_repo",):
    if _p not in sys.path and os.path.isdir(_p):
        sys.path.insert(0, _p)

import ml_dtypes
import numpy as np

import concourse.bass as bass
import concourse.mybir as mybir
import concourse.tile as tile
from concourse import bacc

P = 128
D = 1024          # model dim
N = 2048          # sequence length
HD = 64           # head dim
GH = 4            # heads per core
DG = GH * HD      # 256 projected cols per core
KD = D // P       # 8 k-tiles over model dim
NT = N // P       # 16 tiles over sequence
QC = 512          # n_q chunk size
NQC = N // QC     # 4 chunks
NSTEP = 8 * NT    # 128 pipeline steps
SCALE = HD ** -0.5

F32 = mybir.dt.float32
BF16 = mybir.dt.bfloat16

Exp = mybir.ActivationFunctionType.Exp

# step s -> (block, t); blocks in (chunk, pair) order so out-proj for chunk c
# unlocks as early as possible (after block (c, 1))
BLOCKS = [(c, pr) for c in range(NQC) for pr in range(2)]


def build_nc():
    nc = bacc.Bacc("TRN2")

    xt = nc.declare_dram_parameter("xt", [D, N], BF16, isOutput=False)
    wq = nc.declare_dram_parameter("wq", [D, DG], BF16, isOutput=False)
    wk = nc.declare_dram_parameter("wk", [D, DG], BF16, isOutput=False)
    wv = nc.declare_dram_parameter("wv", [D, DG], BF16, isOutput=False)
    wo = nc.declare_dram_parameter("wo", [DG, D], BF16, isOutput=False)
    y = nc.declare_dram_parameter("y", [N, D], BF16, isOutput=True)

    xt_r = xt[:, :].rearrange("(o p) n -> p o n", p=P)    # [128, 8, 2048]
    wq_r = wq[:, :].rearrange("(o p) n -> p o n", p=P)    # [128, 8, 256]
    wk_r = wk[:, :].rearrange("(o p) n -> p o n", p=P)
    wv_r = wv[:, :].rearrange("(o p) n -> p o n", p=P)
    wo_r = wo[:, :].rearrange("(o p) n -> p o n", p=P)    # [128, 2, 1024]
    y_r = y[:, :].rearrange("(o p) n -> p o n", p=P)      # [128, 16, 1024]

    with tile.TileContext(nc) as tc, nc.allow_low_precision("bf16 attention"):
        with (
            tc.tile_pool(name="wpool", bufs=1) as wpool,
            tc.tile_pool(name="qkvpool", bufs=1) as qkvpool,
            tc.tile_pool(name="xpool", bufs=1) as xpool,
            tc.tile_pool(name="epool", bufs=6) as epool,
            tc.tile_pool(name="work", bufs=2) as work,
            tc.tile_pool(name="opool", bufs=4) as opool,
            tc.tile_pool(name="rcpool", bufs=4) as rcpool,
            tc.tile_pool(name="outp", bufs=2) as outp,
            tc.tile_pool(name="ps_st", bufs=2, space="PSUM") as ps_st,
            tc.tile_pool(name="ps_o", bufs=2, space="PSUM") as ps_o,
            tc.tile_pool(name="ps_f", bufs=2, space="PSUM") as ps_f,
        ):
            # ---------------- input DMAs, in consumption order ----------------
            wk_sb = wpool.tile([P, KD, DG], BF16, tag="wk")
            wq_sb = wpool.tile([P, KD, DG], BF16, tag="wq")
            wv_sb = wpool.tile([P, KD, DG], BF16, tag="wv")
            wo_sb = wpool.tile([P, 2, D], BF16, tag="wo")
            xt_sb = xpool.tile([P, KD, N], BF16, tag="xt")

            # few BIG 3D DMAs (each DMA_DIRECT2D occupies its issuing
            # queue ~0.65us, so issue count matters).  The bulk of x is
            # pinned BEHIND the critical first 2.5MB (wk+xc0+wq+wv) with an
            # explicit dependency so it doesn't steal HBM bandwidth from the
            # transfers that gate the first exp.
            # staged, ring-parallel input DMAs.  One DMA ring moves only
            # ~100-200GB/s, so the first-exp-critical 2MB (wk+xc0+wq) is
            # split across ~6 rings; later stages are gated behind earlier
            # ones with explicit deps so they never steal HBM bandwidth
            # from the transfer that gates the pipeline next.
            s1 = [
                nc.sync.dma_start(xt_sb[:, 0:2, 0:QC], xt_r[:, 0:2, 0:QC]),
                nc.sync.dma_start(xt_sb[:, 2:4, 0:QC], xt_r[:, 2:4, 0:QC]),
                nc.sync.dma_start(xt_sb[:, 4:6, 0:QC], xt_r[:, 4:6, 0:QC]),
                nc.sync.dma_start(xt_sb[:, 6:KD, 0:QC], xt_r[:, 6:KD, 0:QC]),
                nc.sync.dma_start(wk_sb[:, :, 0:P], wk_r[:, :, 0:P]),
            ]
            s2 = [
                nc.sync.dma_start(wq_sb[:, :, 0:P], wq_r[:, :, 0:P]),
                nc.sync.dma_start(wv_sb[:], wv_r),
            ]
            s3 = [
                nc.gpsimd.dma_start(xt_sb[:, :, QC:8 * P], xt_r[:, :, QC:8 * P]),
                nc.gpsimd.dma_start(wk_sb[:, :, P:DG], wk_r[:, :, P:DG]),
                nc.gpsimd.dma_start(wq_sb[:, :, P:DG], wq_r[:, :, P:DG]),
            ]
            s4 = [
                nc.gpsimd.dma_start(xt_sb[:, :, 8 * P:12 * P], xt_r[:, :, 8 * P:12 * P]),
            ]
            s5 = [
                nc.gpsimd.dma_start(xt_sb[:, :, 12 * P:N], xt_r[:, :, 12 * P:N]),
                nc.gpsimd.dma_start(wo_sb[:], wo_r),
            ]
            prev = s1
            for stage in (s2, s3, s4, s5):
                for d in stage:
                    for p in prev:
                        bass._add_dep_helper(
                            d.ins, p.ins, sync=True, reason="staged input dma"
                        )
                prev = stage

            # ---------------- persistent tensors ----------------
            qt_sb = qkvpool.tile([P, 2, N], BF16, tag="qt")   # [256, 2048] qT
            kt_sb = qkvpool.tile([P, 2, N], BF16, tag="kt")   # [256, 2048] kT
            vg_sb = qkvpool.tile([P, NT, GH, 66], BF16, tag="vg")  # v + ones
            nc.scalar.copy(
                vg_sb[:, :, :, HD:], nc.const_aps.tensor(1.0, (P, NT, GH, 2), F32)
            )
            at_sb = qkvpool.tile([P, 2, N], BF16, tag="at")   # attn_outT

            # ---------------- filler units ----------------
            # Each filler unit is a list of pieces; a piece is a closure that
            # emits ~2 matmuls (and, on a unit's last piece, the PSUM->SBUF
            # copy / DMA).  Units alternate between the two ps_f banks.

            def kq_unit(which, w_sb, dst, m, cchunk):
                state = {}
                cs = slice(cchunk * QC, (cchunk + 1) * QC)

                def piece(i0):
                    def go():
                        if i0 == 0:
                            state["ps"] = ps_f.tile(
                                [P, QC], F32, tag="f",
                                name=f"{which}ps_{m}_{cchunk}",
                            )
                        ps = state["ps"]
                        for k in range(i0, i0 + 2):
                            nc.tensor.matmul(
                                ps[:],
                                w_sb[:, k, m * P:(m + 1) * P],
                                xt_sb[:, k, cs],
                                start=(k == 0),
                                stop=(k == KD - 1),
                            )
                        if i0 + 2 == KD:
                            nc.vector.tensor_copy(dst[:, m, cs], ps[:])
                    return go

                return [piece(i) for i in range(0, KD, 2)]

            def v_unit(t):
                state = {}

                def piece(i0):
                    def go():
                        if i0 == 0:
                            state["ps"] = ps_f.tile(
                                [P, QC], F32, tag="f", name=f"vps_{t}"
                            )
                        ps = state["ps"]
                        for k in range(i0, i0 + 4):
                            nc.tensor.matmul(
                                ps[:, :DG],
                                xt_sb[:, k, t * P:(t + 1) * P],
                                wv_sb[:, k, :],
                                start=(k == 0),
                                stop=(k == KD - 1),
                            )
                        if i0 + 4 == KD:
                            nc.vector.tensor_copy(
                                vg_sb[:, t, :, 0:HD],
                                ps[:, :DG].rearrange("p (h e) -> p h e", h=GH),
                            )
                    return go

                return [piece(i) for i in range(0, KD, 4)]

            tail_n = [0]

            def proj_unit(m, nn, tail=False):
                def go():
                    if tail:
                        # ps_o's banks are free once the last block is
                        # staged; alternating pools gives the tail proj a
                        # 4-deep PSUM rotation
                        pool = ps_o if tail_n[0] % 2 else ps_f
                        tail_n[0] += 1
                        ps = pool.tile(
                            [P, QC], F32, tag="o" if pool is ps_o else "f",
                            name=f"yps_{m}_{nn}",
                        )
                    else:
                        ps = ps_f.tile([P, QC], F32, tag="f", name=f"yps_{m}_{nn}")
                    for ks in range(2):
                        nc.tensor.matmul(
                            ps[:],
                            at_sb[:, ks, m * P:(m + 1) * P],
                            wo_sb[:, ks, nn * QC:(nn + 1) * QC],
                            start=(ks == 0),
                            stop=(ks == 1),
                        )
                    ysb = outp.tile([P, QC], BF16, tag="y", name=f"y_{m}_{nn}")
                    if tail:
                        # ScalarE is idle once the last exp is done, while
                        # DVE is busy with the tail reciprocals
                        nc.scalar.copy(ysb[:], ps[:])
                    else:
                        nc.vector.tensor_copy(ysb[:], ps[:])
                    if tail:
                        nc.gpsimd.dma_start(y_r[:, m, nn * QC:(nn + 1) * QC], ysb[:])
                    else:
                        nc.sync.dma_start(y_r[:, m, nn * QC:(nn + 1) * QC], ysb[:])
                return [go]

            # (deadline, pieces) — deadline = step by which the unit should be
            # emitted (need-step minus ~2-3 of execution-lag lead time); proj
            # units are appended with deadline None (pace-driven)
            fillers = []
            for t in range(1, NT):
                fillers.append((t - 2, v_unit(t)))
            for cc in range(1, NQC):
                fillers.append((4 * cc - 4, kq_unit("k", wk_sb, kt_sb, 0, cc)))
            for cc in range(NQC):
                fillers.append((12 + 4 * cc, kq_unit("k", wk_sb, kt_sb, 1, cc)))
            fillers.append((12, kq_unit("q", wq_sb, qt_sb, 1, 0)))
            for cc in range(1, NQC):
                fillers.append((32 * cc - 4, kq_unit("q", wq_sb, qt_sb, 0, cc)))
                fillers.append((32 * cc + 12, kq_unit("q", wq_sb, qt_sb, 1, cc)))
            fillers.sort(key=lambda f: f[0])
            fill_q = [(d, p) for d, pieces in fillers for p in pieces]
            total_pieces = len(fill_q) + 32  # + proj pieces appended later
            emitted = [0]

            def emit_fillers(s):
                # everything whose deadline has arrived, then pace the rest
                while fill_q and fill_q[0][0] is not None and fill_q[0][0] <= s:
                    fill_q.pop(0)[1]()
                    emitted[0] += 1
                pace = (s + 1) * total_pieces // NSTEP
                while fill_q and emitted[0] < pace:
                    fill_q.pop(0)[1]()
                    emitted[0] += 1

            # ---------------- attention stream helpers ----------------
            st_tiles = {}
            e_tiles = {}

            def emit_sc(s):
                b, t = s // NT, s % NT
                c, pr = BLOCKS[b]
                cs = slice(c * QC, (c + 1) * QC)
                ts_ = slice(t * P, (t + 1) * P)
                st = ps_st.tile([P, 2, QC], F32, tag="st", name=f"st_{s}")
                for half in range(2):
                    hs = slice(half * HD, (half + 1) * HD)
                    nc.tensor.matmul(
                        st[:, half, :],
                        kt_sb[hs, pr, ts_],
                        qt_sb[hs, pr, cs],
                        start=True,
                        stop=True,
                    )
                st_tiles[s] = st

            def emit_exp(s):
                e = epool.tile([P, 2, QC], BF16, tag="e", name=f"e_{s}")
                nc.scalar.activation(e[:], st_tiles.pop(s)[:], Exp, scale=SCALE)
                e_tiles[s] = e

            def emit_pv(s, o_ps):
                b, t = s // NT, s % NT
                _, pr = BLOCKS[b]
                e = e_tiles.pop(s)
                for half in range(2):
                    h = 2 * pr + half
                    nc.tensor.matmul(
                        o_ps[half][:],
                        vg_sb[:, t, h, 0:HD + 1],
                        e[:, half, :],
                        start=(t == 0),
                        stop=(t == NT - 1),
                    )

            # ---------------- softmax normalization ----------------
            # Per-block: stage the two [65,512] accumulators to SBUF (frees
            # the PSUM banks) and DMA-reshape the denominator rows [1,512]
            # into partition-spread [32,16] slots of the chunk's den tile.
            # Per-chunk: ONE reciprocal over [32,4,16] (iterative-divide cost
            # scales with per-lane elements: 64 here vs 512 for a row),
            # DMA-reshape each slot back to a [1,512] row, partition-
            # broadcast on GpSimd, multiply into A^T on DVE.
            chunk_state = {}

            def emit_stage_and_norm(b):
                # stage this block's accumulators, spread its two denominator
                # rows across partitions, reciprocal, and normalize this
                # pair's A^T rows right away (per-pair, so only half the
                # chain lands after the final block)
                c, pr = BLOCKS[b]
                if pr == 0:
                    den = work.tile([32, 4, 16], F32, tag="den", name=f"den_{c}")
                    rcT = work.tile([32, 4, 16], F32, tag="rcT", name=f"rcT_{c}")
                    chunk_state[c] = {"den": den, "rcT": rcT}
                st_ = chunk_state[c]
                den, rcT = st_["den"], st_["rcT"]
                o_sbs = []
                for half in range(2):
                    o_sb = opool.tile(
                        [HD + 1, QC], F32, tag="osb", name=f"osb_{b}_{half}"
                    )
                    nc.vector.tensor_copy(o_sb[:], o_tiles[b][half][:])
                    o_sbs.append(o_sb)
                    slot = 2 * pr + half
                    nc.sync.dma_start(den[:, slot, :], o_sb[HD:HD + 1, :])
                nc.vector.reciprocal(
                    rcT[:, 2 * pr:2 * pr + 2, :], den[:, 2 * pr:2 * pr + 2, :]
                )
                for half in range(2):
                    o_sb = o_sbs[half]
                    slot = 2 * pr + half
                    rcr = rcpool.tile(
                        [1, QC], F32, tag="rcr", name=f"rcr_{c}_{slot}"
                    )
                    nc.sync.dma_start(rcr[:], rcT[:, slot, :])
                    rbs = rcpool.tile(
                        [HD, QC], F32, tag="rbs", name=f"rbs_{c}_{slot}"
                    )
                    nc.gpsimd.partition_broadcast(rbs[:], rcr[:])
                    dst = slice(c * QC, (c + 1) * QC)
                    if half == 0:
                        nc.vector.tensor_mul(
                            at_sb[0:HD, pr, dst], o_sb[0:HD, :], rbs[:]
                        )
                    else:
                        stg = work.tile(
                            [HD, QC], BF16, tag="stg", name=f"stg_{c}_{pr}"
                        )
                        nc.vector.tensor_mul(stg[:], o_sb[0:HD, :], rbs[:])
                        nc.sync.dma_start(at_sb[HD:P, pr, dst], stg[:])
                if pr == 1:
                    chunk_state.pop(c)

            # ---------------- prologue ----------------
            # ~3.5us of dummy matmuls on constants while the first DMAs are
            # in flight: the PE's HAM clock-gate needs ~3.4us of sustained
            # activity to unthrottle 1.2->2.4GHz, so the first REAL matmuls
            # run at full speed instead of paying the cold tax
            warm_c = nc.const_aps.tensor(1.0, (P, QC), BF16)
            for wi in range(8):
                wps = ps_f.tile([P, QC], F32, tag="f", name=f"warm_{wi}")
                nc.tensor.matmul(
                    wps[:], warm_c[:, 0:P], warm_c[:], start=True, stop=True
                )
            for p in kq_unit("k", wk_sb, kt_sb, 0, 0):
                p()
            for p in kq_unit("q", wq_sb, qt_sb, 0, 0):
                p()
            for p in v_unit(0):
                p()

            # ---------------- the 128-step stream ----------------
            o_tiles = {}

            def alloc_o(b):
                o_ps = []
                for half in range(2):
                    o_full = ps_o.tile([P, QC], F32, tag="o", name=f"o_{b}_{half}")
                    o_ps.append(o_full[: HD + 1])
                o_tiles[b] = o_ps

            emit_sc(0)
            for s in range(NSTEP):
                b, t = s // NT, s % NT
                if t == 0:
                    alloc_o(b)
                if s + 1 < NSTEP:
                    emit_sc(s + 1)
                emit_exp(s)
                emit_pv(s, o_tiles[b])
                if t == NT - 1:
                    c, pr = BLOCKS[b]
                    if b < len(BLOCKS) - 1:
                        emit_stage_and_norm(b)
                        if pr == 1:
                            for mi in range(4):
                                for nn in range(2):
                                    fill_q.append(
                                        (None, proj_unit(4 * c + mi, nn)[0])
                                    )
                emit_fillers(s)
            # tail: last block's normalization via the direct (low-latency)
            # reciprocal path, column-split and interleaved with the last
            # chunk's proj units so the PE restarts as early as possible
            b = len(BLOCKS) - 1
            c, pr = BLOCKS[b]
            o_sbs = []
            for half in range(2):
                o_sb = opool.tile(
                    [HD + 1, QC], F32, tag="osb", name=f"osb_{b}_{half}"
                )
                # ScalarE is idle after the last exp; keep DVE free for the
                # reciprocals
                nc.scalar.copy(o_sb[:], o_tiles[b][half][:])
                o_sbs.append(o_sb)
            for sp in range(2):
                w = QC // 2
                ss = slice(sp * w, (sp + 1) * w)
                for half in range(2):
                    o_sb = o_sbs[half]
                    rc = rcpool.tile([1, w], F32, tag="rcd", name=f"rcd_{half}_{sp}")
                    nc.vector.reciprocal(rc[:], o_sb[HD:HD + 1, ss])
                    rbs = rcpool.tile([HD, w], F32, tag="rbsd", name=f"rbsd_{half}_{sp}")
                    nc.gpsimd.partition_broadcast(rbs[:], rc[:])
                    dst = slice(c * QC + sp * w, c * QC + (sp + 1) * w)
                    if half == 0:
                        nc.vector.tensor_mul(
                            at_sb[0:HD, pr, dst], o_sb[0:HD, ss], rbs[:]
                        )
                    else:
                        stg = work.tile(
                            [HD, w], BF16, tag="stgd", name=f"stgd_{sp}"
                        )
                        nc.vector.tensor_mul(stg[:], o_sb[0:HD, ss], rbs[:])
                        nc.sync.dma_start(at_sb[HD:P, pr, dst], stg[:])
                for mi in (2 * sp, 2 * sp + 1):
                    for nn in range(2):
                        fill_q.append(
                            (None, proj_unit(4 * c + mi, nn, tail=True)[0])
                        )
                while fill_q:
                    fill_q.pop(0)[1]()

    nc.finalize()
    return nc


_NC = None


def _get_nc():
    global _NC
    if _NC is None:
        _NC = build_nc()
    return _NC


def _in_maps(x, w_qkv, w_out):
    bf = ml_dtypes.bfloat16
    x = np.asarray(x, dtype=np.float32)
    w_qkv = np.asarray(w_qkv, dtype=np.float32)
    w_out = np.asarray(w_out, dtype=np.float32)
    xts = [np.ascontiguousarray(x[b].T).astype(bf) for b in range(2)]
    wq_g = [np.ascontiguousarray(w_qkv[:, 0 * D + g * DG:0 * D + (g + 1) * DG]).astype(bf) for g in range(4)]
    wk_g = [np.ascontiguousarray(w_qkv[:, 1 * D + g * DG:1 * D + (g + 1) * DG]).astype(bf) for g in range(4)]
    wv_g = [np.ascontiguousarray(w_qkv[:, 2 * D + g * DG:2 * D + (g + 1) * DG]).astype(bf) for g in range(4)]
    wo_g = [np.ascontiguousarray(w_out[g * DG:(g + 1) * DG, :]).astype(bf) for g in range(4)]
    maps = []
    for c in range(8):
        b, g = c // 4, c % 4
        maps.append({
            "xt": xts[b],
            "wq": wq_g[g],
            "wk": wk_g[g],
            "wv": wv_g[g],
            "wo": wo_g[g],
        })
    return maps


LAST_RESULT = None


def kernel(x, w_qkv, w_out, b_out):
    from concourse.bass_utils import run_bass_kernel_spmd

    nc = _get_nc()
    maps = _in_maps(x, w_qkv, w_out)
    res = run_bass_kernel_spmd(nc, maps, list(range(8)))
    global LAST_RESULT
    LAST_RESULT = res
    out = np.zeros((2, N, D), dtype=np.float32)
    for c in range(8):
        out[c // 4] += np.asarray(res.results[c]["y"], dtype=np.float32)
    out += np.asarray(b_out, dtype=np.float32)[None, None, :]
    return out


# revision 5
# speedup vs baseline: 1.0447x; 1.0027x over previous
"""Multi-head attention (b=2, n=2048, d=1024, H=16 heads) on 8 TRN2 NeuronCores.

Sharding: core c = (b, g) with b = c // 4 (data parallel over batch) and
g = c % 4 (tensor parallel over head groups of 4 heads).  Each core computes
qkv projections for its 4 heads, full softmax attention for those heads, and
a partial output projection y_partial = A_heads @ w_out[g*256:(g+1)*256].
The host sums the 4 partials per batch (bf16 partials, f32 accumulate) and
adds b_out.

v2 redesign: the kernel is one flat software-pipelined stream of 128 steps
(8 blocks of (q-chunk, head-pair) x 16 nk-tiles).  Per step s the PE emission
order is [scores(s+1) pair, PV0(s), PV1(s), filler pieces]; the exp for step
s runs on ScalarE (the only engine with Exp; ~1.0us per [128,1024] tile).
Scores(s+1) depends only on exp(s-1) (PSUM WAR through the 2-deep st
rotation), so ScalarE runs back-to-back while the PE fills its exp-wait with
the score pair for the next step plus one piece of projection work
(qkv / out-proj), keeping both engines busy concurrently.

PSUM budget (8 banks): st double-buffer 2x2 + o accumulators 2 + filler 2.
Fillers (kq units 8mm/4 pieces, v units 8mm/2 pieces, proj units 2mm) are
emitted deadline-first then paced uniformly; the greedy tile scheduler pops
them whenever the PE would otherwise idle.

Softmax denominators: DVE reciprocal is an iterative ~8-cycle/element op
whose cost scales with per-lane (free-dim) element count, so reciprocal of a
[1,512] denominator row costs ~2us and head-of-line-blocks the DVE queue
(which stalls the kq-unit copies the next block's scores wait on).  Instead,
each chunk's 4 denominator rows are DMA-reshaped into a [32,4,16] tile
(elements spread across partitions), reciprocal'd in ONE ~0.7us op, DMA-
reshaped back to [1,512] rows, partition-broadcast on the (otherwise idle)
GpSimd engine, and multiplied into A^T bf16 on DVE.  half1 of each pair is
shifted to partitions 64-127 via SBUF->SBUF DMA.

y is stored bf16 (halves both the PSUM->SBUF copy cost and the HBM write);
the host accumulates partials in f32.  Measured end-to-end relative error
~5e-3 vs the fp32 reference.
"""

import os
import sys

for _p in ("/opt/trn_rl_repo",):
    if _p not in sys.path and os.path.isdir(_p):
        sys.path.insert(0, _p)

import ml_dtypes
import numpy as np

import concourse.bass as bass
import concourse.mybir as mybir
import concourse.tile as tile
from concourse import bacc

P = 128
D = 1024          # model dim
N = 2048          # sequence length
HD = 64           # head dim
GH = 4            # heads per core
DG = GH * HD      # 256 projected cols per core
KD = D // P       # 8 k-tiles over model dim
NT = N // P       # 16 tiles over sequence
QC = 512          # n_q chunk size
NQC = N // QC     # 4 chunks
NSTEP = 8 * NT    # 128 pipeline steps
SCALE = HD ** -0.5

F32 = mybir.dt.float32
BF16 = mybir.dt.bfloat16

Exp = mybir.ActivationFunctionType.Exp

# step s -> (block, t).  The first two blocks are pr=0 so the kt-m1/qt-m1
# projection units shift out of the overloaded first 32 steps (which must
# already absorb all 16 v units) into the filler-starved middle; from chunk 2
# on, (c,0),(c,1) pairs keep out-proj unlocking steadily.
BLOCKS = [(0, 0), (1, 0), (0, 1), (1, 1), (2, 0), (2, 1), (3, 0), (3, 1)]


def build_nc():
    nc = bacc.Bacc("TRN2")

    xt = nc.declare_dram_parameter("xt", [D, N], BF16, isOutput=False)
    wq = nc.declare_dram_parameter("wq", [D, DG], BF16, isOutput=False)
    wk = nc.declare_dram_parameter("wk", [D, DG], BF16, isOutput=False)
    wv = nc.declare_dram_parameter("wv", [D, DG], BF16, isOutput=False)
    wo = nc.declare_dram_parameter("wo", [DG, D], BF16, isOutput=False)
    y = nc.declare_dram_parameter("y", [N, D], BF16, isOutput=True)

    xt_r = xt[:, :].rearrange("(o p) n -> p o n", p=P)    # [128, 8, 2048]
    wq_r = wq[:, :].rearrange("(o p) n -> p o n", p=P)    # [128, 8, 256]
    wk_r = wk[:, :].rearrange("(o p) n -> p o n", p=P)
    wv_r = wv[:, :].rearrange("(o p) n -> p o n", p=P)
    wo_r = wo[:, :].rearrange("(o p) n -> p o n", p=P)    # [128, 2, 1024]
    y_r = y[:, :].rearrange("(o p) n -> p o n", p=P)      # [128, 16, 1024]

    with tile.TileContext(nc) as tc, nc.allow_low_precision("bf16 attention"):
        with (
            tc.tile_pool(name="wpool", bufs=1) as wpool,
            tc.tile_pool(name="qkvpool", bufs=1) as qkvpool,
            tc.tile_pool(name="xpool", bufs=1) as xpool,
            tc.tile_pool(name="epool", bufs=6) as epool,
            tc.tile_pool(name="work", bufs=2) as work,
            tc.tile_pool(name="opool", bufs=4) as opool,
            tc.tile_pool(name="rcpool", bufs=4) as rcpool,
            tc.tile_pool(name="outp", bufs=2) as outp,
            tc.tile_pool(name="ps_st", bufs=2, space="PSUM") as ps_st,
            tc.tile_pool(name="ps_o", bufs=2, space="PSUM") as ps_o,
            tc.tile_pool(name="ps_f", bufs=2, space="PSUM") as ps_f,
        ):
            # ---------------- input DMAs, in consumption order ----------------
            wk_sb = wpool.tile([P, KD, DG], BF16, tag="wk")
            wq_sb = wpool.tile([P, KD, DG], BF16, tag="wq")
            wv_sb = wpool.tile([P, KD, DG], BF16, tag="wv")
            wo_sb = wpool.tile([P, 2, D], BF16, tag="wo")
            xt_sb = xpool.tile([P, KD, N], BF16, tag="xt")

            # few BIG 3D DMAs (each DMA_DIRECT2D occupies its issuing
            # queue ~0.65us, so issue count matters).  The bulk of x is
            # pinned BEHIND the critical first 2.5MB (wk+xc0+wq+wv) with an
            # explicit dependency so it doesn't steal HBM bandwidth from the
            # transfers that gate the first exp.
            # staged, ring-parallel input DMAs.  One DMA ring moves only
            # ~100-200GB/s, so the first-exp-critical 2MB (wk+xc0+wq) is
            # split across ~6 rings; later stages are gated behind earlier
            # ones with explicit deps so they never steal HBM bandwidth
            # from the transfer that gates the pipeline next.
            s1 = [
                nc.sync.dma_start(xt_sb[:, 0:2, 0:QC], xt_r[:, 0:2, 0:QC]),
                nc.sync.dma_start(xt_sb[:, 2:4, 0:QC], xt_r[:, 2:4, 0:QC]),
                nc.sync.dma_start(xt_sb[:, 4:6, 0:QC], xt_r[:, 4:6, 0:QC]),
                nc.sync.dma_start(xt_sb[:, 6:KD, 0:QC], xt_r[:, 6:KD, 0:QC]),
                nc.sync.dma_start(wk_sb[:, :, 0:P], wk_r[:, :, 0:P]),
            ]
            s2 = [
                nc.sync.dma_start(wq_sb[:, :, 0:P], wq_r[:, :, 0:P]),
                nc.sync.dma_start(wv_sb[:], wv_r),
            ]
            s3 = [
                nc.gpsimd.dma_start(xt_sb[:, :, QC:8 * P], xt_r[:, :, QC:8 * P]),
                nc.gpsimd.dma_start(wk_sb[:, :, P:DG], wk_r[:, :, P:DG]),
                nc.gpsimd.dma_start(wq_sb[:, :, P:DG], wq_r[:, :, P:DG]),
            ]
            s4 = [
                nc.gpsimd.dma_start(xt_sb[:, :, 8 * P:12 * P], xt_r[:, :, 8 * P:12 * P]),
            ]
            s5 = [
                nc.gpsimd.dma_start(xt_sb[:, :, 12 * P:N], xt_r[:, :, 12 * P:N]),
                nc.gpsimd.dma_start(wo_sb[:], wo_r),
            ]
            prev = s1
            for stage in (s2, s3, s4, s5):
                for d in stage:
                    for p in prev:
                        bass._add_dep_helper(
                            d.ins, p.ins, sync=True, reason="staged input dma"
                        )
                prev = stage

            # ---------------- persistent tensors ----------------
            qt_sb = qkvpool.tile([P, 2, N], BF16, tag="qt")   # [256, 2048] qT
            kt_sb = qkvpool.tile([P, 2, N], BF16, tag="kt")   # [256, 2048] kT
            vg_sb = qkvpool.tile([P, NT, GH, 66], BF16, tag="vg")  # v + ones
            nc.scalar.copy(
                vg_sb[:, :, :, HD:], nc.const_aps.tensor(1.0, (P, NT, GH, 2), F32)
            )
            at_sb = qkvpool.tile([P, 2, N], BF16, tag="at")   # attn_outT

            # ---------------- filler units ----------------
            # Each filler unit is a list of pieces; a piece is a closure that
            # emits ~2 matmuls (and, on a unit's last piece, the PSUM->SBUF
            # copy / DMA).  Units alternate between the two ps_f banks.

            def kq_unit(which, w_sb, dst, m, cchunk):
                state = {}
                cs = slice(cchunk * QC, (cchunk + 1) * QC)

                def piece(i0):
                    def go():
                        if i0 == 0:
                            state["ps"] = ps_f.tile(
                                [P, QC], F32, tag="f",
                                name=f"{which}ps_{m}_{cchunk}",
                            )
                        ps = state["ps"]
                        for k in range(i0, i0 + 2):
                            nc.tensor.matmul(
                                ps[:],
                                w_sb[:, k, m * P:(m + 1) * P],
                                xt_sb[:, k, cs],
                                start=(k == 0),
                                stop=(k == KD - 1),
                            )
                        if i0 + 2 == KD:
                            nc.vector.tensor_copy(dst[:, m, cs], ps[:])
                    return go

                return [piece(i) for i in range(0, KD, 2)]

            def v_unit(t):
                state = {}

                def piece(i0):
                    def go():
                        if i0 == 0:
                            state["ps"] = ps_f.tile(
                                [P, QC], F32, tag="f", name=f"vps_{t}"
                            )
                        ps = state["ps"]
                        for k in range(i0, i0 + 4):
                            nc.tensor.matmul(
                                ps[:, :DG],
                                xt_sb[:, k, t * P:(t + 1) * P],
                                wv_sb[:, k, :],
                                start=(k == 0),
                                stop=(k == KD - 1),
                            )
                        if i0 + 4 == KD:
                            nc.vector.tensor_copy(
                                vg_sb[:, t, :, 0:HD],
                                ps[:, :DG].rearrange("p (h e) -> p h e", h=GH),
                            )
                    return go

                return [piece(i) for i in range(0, KD, 4)]

            tail_n = [0]

            def proj_unit(m, nn, tail=False):
                def go():
                    if tail:
                        # ps_o's and ps_st's banks are free once the last
                        # block is staged / exp'd; cycling pools gives the
                        # tail proj a 6-deep PSUM rotation
                        pool, tg = [(ps_f, "f"), (ps_o, "o"), (ps_st, "st")][
                            tail_n[0] % 3
                        ]
                        tail_n[0] += 1
                        ps = pool.tile(
                            [P, QC], F32, tag=tg, name=f"yps_{m}_{nn}"
                        )
                    else:
                        ps = ps_f.tile([P, QC], F32, tag="f", name=f"yps_{m}_{nn}")
                    for ks in range(2):
                        nc.tensor.matmul(
                            ps[:],
                            at_sb[:, ks, m * P:(m + 1) * P],
                            wo_sb[:, ks, nn * QC:(nn + 1) * QC],
                            start=(ks == 0),
                            stop=(ks == 1),
                        )
                    ysb = outp.tile([P, QC], BF16, tag="y", name=f"y_{m}_{nn}")
                    if tail and tail_n[0] % 2 == 0:
                        # ScalarE and (after the reciprocals) DVE are both
                        # idle at the tail; alternating halves the cast chain
                        nc.scalar.copy(ysb[:], ps[:])
                    else:
                        nc.vector.tensor_copy(ysb[:], ps[:])
                    if tail:
                        nc.gpsimd.dma_start(y_r[:, m, nn * QC:(nn + 1) * QC], ysb[:])
                    else:
                        nc.sync.dma_start(y_r[:, m, nn * QC:(nn + 1) * QC], ysb[:])
                return [go]

            # (deadline, pieces) — deadline = step by which the unit should be
            # emitted (need-step minus ~2-3 of execution-lag lead time); proj
            # units are appended with deadline None (pace-driven)
            fillers = []
            for t in range(1, NT):
                fillers.append((t - 2, v_unit(t)))
            for cc in range(1, NQC):
                fillers.append((4 * cc - 4, kq_unit("k", wk_sb, kt_sb, 0, cc)))
            fillers.append((10, kq_unit("q", wq_sb, qt_sb, 0, 1)))
            for cc in range(NQC):
                fillers.append((26 + 4 * cc, kq_unit("k", wk_sb, kt_sb, 1, cc)))
            fillers.append((26, kq_unit("q", wq_sb, qt_sb, 1, 0)))
            fillers.append((42, kq_unit("q", wq_sb, qt_sb, 1, 1)))
            fillers.append((58, kq_unit("q", wq_sb, qt_sb, 0, 2)))
            fillers.append((74, kq_unit("q", wq_sb, qt_sb, 1, 2)))
            fillers.append((90, kq_unit("q", wq_sb, qt_sb, 0, 3)))
            fillers.append((106, kq_unit("q", wq_sb, qt_sb, 1, 3)))
            fillers.sort(key=lambda f: f[0])
            fill_q = [(d, p) for d, pieces in fillers for p in pieces]
            total_pieces = len(fill_q) + 32  # + proj pieces appended later
            emitted = [0]

            def emit_fillers(s):
                # everything whose deadline has arrived, then pace the rest
                while fill_q and fill_q[0][0] is not None and fill_q[0][0] <= s:
                    fill_q.pop(0)[1]()
                    emitted[0] += 1
                pace = (s + 1) * total_pieces // NSTEP
                while fill_q and emitted[0] < pace:
                    fill_q.pop(0)[1]()
                    emitted[0] += 1

            # ---------------- attention stream helpers ----------------
            st_tiles = {}
            e_tiles = {}

            def emit_sc(s):
                b, t = s // NT, s % NT
                c, pr = BLOCKS[b]
                cs = slice(c * QC, (c + 1) * QC)
                ts_ = slice(t * P, (t + 1) * P)
                st = ps_st.tile([P, 2, QC], F32, tag="st", name=f"st_{s}")
                for half in range(2):
                    hs = slice(half * HD, (half + 1) * HD)
                    nc.tensor.matmul(
                        st[:, half, :],
                        kt_sb[hs, pr, ts_],
                        qt_sb[hs, pr, cs],
                        start=True,
                        stop=True,
                    )
                st_tiles[s] = st

            def emit_exp(s):
                e = epool.tile([P, 2, QC], BF16, tag="e", name=f"e_{s}")
                nc.scalar.activation(e[:], st_tiles.pop(s)[:], Exp, scale=SCALE)
                e_tiles[s] = e

            def emit_pv(s, o_ps):
                b, t = s // NT, s % NT
                _, pr = BLOCKS[b]
                e = e_tiles.pop(s)
                for half in range(2):
                    h = 2 * pr + half
                    nc.tensor.matmul(
                        o_ps[half][:],
                        vg_sb[:, t, h, 0:HD + 1],
                        e[:, half, :],
                        start=(t == 0),
                        stop=(t == NT - 1),
                    )

            # ---------------- softmax normalization ----------------
            # Per-block: stage the two [65,512] accumulators to SBUF (frees
            # the PSUM banks) and DMA-reshape the denominator rows [1,512]
            # into partition-spread [32,16] slots of the chunk's den tile.
            # Per-chunk: ONE reciprocal over [32,4,16] (iterative-divide cost
            # scales with per-lane elements: 64 here vs 512 for a row),
            # DMA-reshape each slot back to a [1,512] row, partition-
            # broadcast on GpSimd, multiply into A^T on DVE.
            chunk_state = {}

            def emit_stage_and_norm(b):
                # stage this block's accumulators, spread its two denominator
                # rows across partitions, reciprocal, and normalize this
                # pair's A^T rows right away (per-pair, so only half the
                # chain lands after the final block)
                c, pr = BLOCKS[b]
                if pr == 0:
                    den = work.tile([32, 4, 16], F32, tag="den", name=f"den_{c}")
                    rcT = work.tile([32, 4, 16], F32, tag="rcT", name=f"rcT_{c}")
                    chunk_state[c] = {"den": den, "rcT": rcT}
                st_ = chunk_state[c]
                den, rcT = st_["den"], st_["rcT"]
                o_sbs = []
                for half in range(2):
                    o_sb = opool.tile(
                        [HD + 1, QC], F32, tag="osb", name=f"osb_{b}_{half}"
                    )
                    nc.vector.tensor_copy(o_sb[:], o_tiles[b][half][:])
                    o_sbs.append(o_sb)
                    slot = 2 * pr + half
                    nc.sync.dma_start(den[:, slot, :], o_sb[HD:HD + 1, :])
                nc.vector.reciprocal(
                    rcT[:, 2 * pr:2 * pr + 2, :], den[:, 2 * pr:2 * pr + 2, :]
                )
                for half in range(2):
                    o_sb = o_sbs[half]
                    slot = 2 * pr + half
                    rcr = rcpool.tile(
                        [1, QC], F32, tag="rcr", name=f"rcr_{c}_{slot}"
                    )
                    nc.sync.dma_start(rcr[:], rcT[:, slot, :])
                    rbs = rcpool.tile(
                        [HD, QC], F32, tag="rbs", name=f"rbs_{c}_{slot}"
                    )
                    nc.gpsimd.partition_broadcast(rbs[:], rcr[:])
                    dst = slice(c * QC, (c + 1) * QC)
                    if half == 0:
                        nc.vector.tensor_mul(
                            at_sb[0:HD, pr, dst], o_sb[0:HD, :], rbs[:]
                        )
                    else:
                        stg = work.tile(
                            [HD, QC], BF16, tag="stg", name=f"stg_{c}_{pr}"
                        )
                        nc.vector.tensor_mul(stg[:], o_sb[0:HD, :], rbs[:])
                        nc.sync.dma_start(at_sb[HD:P, pr, dst], stg[:])
                if pr == 1:
                    chunk_state.pop(c)

            # ---------------- prologue ----------------
            # ~3.5us of dummy matmuls on constants while the first DMAs are
            # in flight: the PE's HAM clock-gate needs ~3.4us of sustained
            # activity to unthrottle 1.2->2.4GHz.  The HAM also RE-throttles
            # when the PE sits mostly idle within its 3.4us window, which
            # happens between x-slice arrivals -- so more warmups (into the
            # still-unused score banks) are interleaved between the prologue
            # pieces; the greedy scheduler pops them exactly in the arrival
            # gaps, keeping the first real matmuls at full clock.
            warm_c = nc.const_aps.tensor(1.0, (P, QC), BF16)
            for wi in range(8):
                wps = ps_f.tile([P, QC], F32, tag="f", name=f"warm_{wi}")
                nc.tensor.matmul(
                    wps[:], warm_c[:, 0:P], warm_c[:], start=True, stop=True
                )

            warm_i = [0]

            def st_warm():
                wps = ps_st.tile(
                    [P, QC], F32, tag="st", name=f"stwarm_{warm_i[0]}"
                )
                warm_i[0] += 1
                nc.tensor.matmul(
                    wps[:], warm_c[:, 0:P], warm_c[:], start=True, stop=True
                )

            for p in kq_unit("k", wk_sb, kt_sb, 0, 0):
                p()
                st_warm()
                st_warm()
            for p in kq_unit("q", wq_sb, qt_sb, 0, 0):
                p()
                st_warm()
            for p in v_unit(0):
                p()

            # ---------------- the 128-step stream ----------------
            o_tiles = {}

            def alloc_o(b):
                o_ps = []
                for half in range(2):
                    o_full = ps_o.tile([P, QC], F32, tag="o", name=f"o_{b}_{half}")
                    o_ps.append(o_full[: HD + 1])
                o_tiles[b] = o_ps

            emit_sc(0)
            for s in range(NSTEP):
                b, t = s // NT, s % NT
                if t == 0:
                    alloc_o(b)
                if s + 1 < NSTEP:
                    emit_sc(s + 1)
                emit_exp(s)
                emit_pv(s, o_tiles[b])
                if t == NT - 1:
                    c, pr = BLOCKS[b]
                    if b < len(BLOCKS) - 1:
                        emit_stage_and_norm(b)
                        if pr == 1:
                            for mi in range(4):
                                for nn in range(2):
                                    fill_q.append(
                                        (None, proj_unit(4 * c + mi, nn)[0])
                                    )
                emit_fillers(s)
            # tail: last block's normalization via the direct (low-latency)
            # reciprocal path, column-split and interleaved with the last
            # chunk's proj units so the PE restarts as early as possible
            b = len(BLOCKS) - 1
            c, pr = BLOCKS[b]
            o_sbs = []
            for half in range(2):
                o_sb = opool.tile(
                    [HD + 1, QC], F32, tag="osb", name=f"osb_{b}_{half}"
                )
                # ScalarE is idle after the last exp; keep DVE free for the
                # reciprocals
                nc.scalar.copy(o_sb[:], o_tiles[b][half][:])
                o_sbs.append(o_sb)
            for sp in range(2):
                w = QC // 2
                ss = slice(sp * w, (sp + 1) * w)
                for half in range(2):
                    o_sb = o_sbs[half]
                    rc = rcpool.tile([1, w], F32, tag="rcd", name=f"rcd_{half}_{sp}")
                    nc.vector.reciprocal(rc[:], o_sb[HD:HD + 1, ss])
                    rbs = rcpool.tile([HD, w], F32, tag="rbsd", name=f"rbsd_{half}_{sp}")
                    nc.gpsimd.partition_broadcast(rbs[:], rc[:])
                    dst = slice(c * QC + sp * w, c * QC + (sp + 1) * w)
                    if half == 0:
                        nc.vector.tensor_mul(
                            at_sb[0:HD, pr, dst], o_sb[0:HD, ss], rbs[:]
                        )
                    else:
                        stg = work.tile(
                            [HD, w], BF16, tag="stgd", name=f"stgd_{sp}"
                        )
                        nc.vector.tensor_mul(stg[:], o_sb[0:HD, ss], rbs[:])
                        nc.sync.dma_start(at_sb[HD:P, pr, dst], stg[:])
                for mi in (2 * sp, 2 * sp + 1):
                    for nn in range(2):
                        fill_q.append(
                            (None, proj_unit(4 * c + mi, nn, tail=True)[0])
                        )
                while fill_q:
                    fill_q.pop(0)[1]()

    nc.finalize()
    return nc


_NC = None


def _get_nc():
    global _NC
    if _NC is None:
        _NC = build_nc()
    return _NC


def _in_maps(x, w_qkv, w_out):
    bf = ml_dtypes.bfloat16
    x = np.asarray(x, dtype=np.float32)
    w_qkv = np.asarray(w_qkv, dtype=np.float32)
    w_out = np.asarray(w_out, dtype=np.float32)
    xts = [np.ascontiguousarray(x[b].T).astype(bf) for b in range(2)]
    wq_g = [np.ascontiguousarray(w_qkv[:, 0 * D + g * DG:0 * D + (g + 1) * DG]).astype(bf) for g in range(4)]
    wk_g = [np.ascontiguousarray(w_qkv[:, 1 * D + g * DG:1 * D + (g + 1) * DG]).astype(bf) for g in range(4)]
    wv_g = [np.ascontiguousarray(w_qkv[:, 2 * D + g * DG:2 * D + (g + 1) * DG]).astype(bf) for g in range(4)]
    wo_g = [np.ascontiguousarray(w_out[g * DG:(g + 1) * DG, :]).astype(bf) for g in range(4)]
    maps = []
    for c in range(8):
        b, g = c // 4, c % 4
        maps.append({
            "xt": xts[b],
            "wq": wq_g[g],
            "wk": wk_g[g],
            "wv": wv_g[g],
            "wo": wo_g[g],
        })
    return maps


LAST_RESULT = None


def kernel(x, w_qkv, w_out, b_out):
    from concourse.bass_utils import run_bass_kernel_spmd

    nc = _get_nc()
    maps = _in_maps(x, w_qkv, w_out)
    res = run_bass_kernel_spmd(nc, maps, list(range(8)))
    global LAST_RESULT
    LAST_RESULT = res
    out = np.zeros((2, N, D), dtype=np.float32)
    for c in range(8):
        out[c // 4] += np.asarray(res.results[c]["y"], dtype=np.float32)
    out += np.asarray(b_out, dtype=np.float32)[None, None, :]
    return out
